# revision 1
# baseline (speedup 1.0000x reference)
"""Causal self-attention on 8 TRN2 NeuronCores.

Sharding: 8 cores = 4 batches x 2 head-groups (data parallel on B,
tensor parallel on heads). Core (b, g) computes batch b, heads
8g..8g+7 end-to-end (qkv slice -> causal attention -> partial
projection); the host sums the two per-batch partials (the "all-reduce
after proj" done host-side since outputs are gathered anyway).

Matmuls run in float32r (TF32 path, full PE rate). Tiles feeding the
PE are float32r-typed so their producers emit the rounding the BIR
verifier requires. The host pre-transposes x and pre-permutes the
weights so every weight DMA is contiguous and no on-device transpose
matmuls are needed: Q^T/K^T come out of the qkv GEMM transposed
(weights stationary), V comes out natural (x^T stationary).

Self-contained: hardcodes B=4, T=2048, C=2048, H=16, HD=128.
"""

import numpy as np

import concourse.bass as bass
import concourse.mybir as mybir
import concourse.tile as tile
from concourse import bacc
from concourse.bass_utils import run_bass_kernel_spmd

B, T, C, H = 4, 2048, 2048, 16
HD = 128          # head dim
G = 2             # head groups (tensor parallel)
HPG = H // G      # 8 heads per core
DG = HPG * HD     # 1024 = per-core concat head dim
N_CORES = 8
SCALE = float(HD) ** -0.5
NEG = -1.0e9      # additive causal mask value

F32 = mybir.dt.float32
F32R = mybir.dt.float32r

P = 128           # partitions
FN = 512          # moving free-dim per matmul (one PSUM bank of fp32)
CI = C // P       # 16 contraction chunks over C
TM = T // P       # 16 t chunks of 128
TN = T // FN      # 4 t chunks of 512
MQK = 2 * DG // P # 16 qk^T row chunks
NV = DG // FN     # 2 v column chunks of 512


def build_nc() -> bass.Bass:
    nc = bacc.Bacc()
    # xt = x.T (host pre-transposed). wqk = [wq|wk] cols for this core's
    # heads, host-permuted to [m, p, ci*128+col]. wv = v cols, host-
    # permuted to [nv, p, ci*512+vc]. wp = w_proj rows, host-permuted to
    # [p, hh, n]. float32r params carry plain fp32 bits.
    xt = nc.declare_dram_parameter("xt", [C, T], F32R, isOutput=False)
    wqk = nc.declare_dram_parameter("wqk", [MQK, P, CI * P], F32R, isOutput=False)
    wv = nc.declare_dram_parameter("wv", [NV, P, CI * FN], F32R, isOutput=False)
    wp = nc.declare_dram_parameter("wp", [P, HPG, C], F32R, isOutput=False)
    masks = nc.declare_dram_parameter("masks", [P, 4, FN], F32, isOutput=False)
    out = nc.declare_dram_parameter("out", [T, C], F32, isOutput=True)

    with tile.TileContext(nc) as tc:
        with (
            tc.tile_pool(name="consts", bufs=1) as consts,
            tc.tile_pool(name="dram", bufs=1, space="DRAM") as dram_pool,
        ):
            ones_f = consts.tile([P, 1], F32)
            nc.gpsimd.memset(ones_f[:], 1.0)
            ones = consts.tile([P, 1], F32R)
            nc.scalar.copy(ones[:], ones_f[:])


            # per-slice DRAM staging so phase C loads only dep on their
            # own producer DMAs (fine-grained B->C overlap)
            qkT_m = [dram_pool.tile([P, T], F32R, name=f"qkT{m}") for m in range(MQK)]
            v_h = [dram_pool.tile([T, HD], F32R, name=f"vh{h}") for h in range(HPG)]
            oT = dram_pool.tile([DG, T], F32)  # attention out^T staging

            # ---------- Phase B: V = x @ wv first, then qk^T = wqk.T @ x.T ----------
            with (
                tc.tile_pool(name="xT", bufs=1) as xT_pool,
            ):
                xT = xT_pool.tile([P, CI, T], F32R)  # x^T resident: 128KB/partition
                for ci in range(CI):
                    nc.sync.dma_start(xT[:, ci, :], xt[ci * P:(ci + 1) * P, :])

                HC = CI // 2  # wv streamed in half-ci tiles
                wq_pool = tc.alloc_tile_pool(name="wq", bufs=2)
                wq_pre = {}
                for m in range(2):  # early FIFO position for the qk weights
                    wq_pre[m] = wq_pool.tile([P, CI, P], F32R, tag="wq", name=f"wqpre{m}")
                    nc.sync.dma_start(
                        wq_pre[m][:], wqk[m, :, :].rearrange("p (ci n) -> p ci n", ci=CI)
                    )
                with (
                    tc.tile_pool(name="wvp", bufs=3) as wv_pool,
                    tc.tile_pool(name="vps", bufs=4, space="PSUM") as vpsum,
                    tc.tile_pool(name="vstage", bufs=4) as vstage,
                ):
                    for nv in range(NV):
                        wvt = {}
                        for half in range(2):
                            t = wv_pool.tile([P, HC, FN], F32R, tag="wvt")
                            nc.sync.dma_start(
                                t[:],
                                wv[nv, :, half * HC * FN:(half + 1) * HC * FN]
                                .rearrange("p (ci n) -> p ci n", ci=HC),
                            )
                            wvt[half] = t
                        for tm in range(TM):
                            ps = vpsum.tile([P, FN], F32)
                            for ci in range(CI):
                                nc.tensor.matmul(
                                    ps[:], xT[:, ci, tm * P:(tm + 1) * P],
                                    wvt[ci // HC][:, ci % HC, :],
                                    start=(ci == 0), stop=(ci == CI - 1),
                                )
                            st = vstage.tile([P, FN], F32R)
                            nc.vector.tensor_copy(st[:], ps[:])
                            for hh in range(4):
                                nc.sync.dma_start(
                                    v_h[4 * nv + hh][tm * P:(tm + 1) * P, :],
                                    st[:, hh * HD:(hh + 1) * HD],
                                )

                with (
                    tc.tile_pool(name="bps", bufs=4, space="PSUM") as bpsum,
                    tc.tile_pool(name="bstage", bufs=4) as bstage,
                ):
                    for m in range(MQK):
                        if m in wq_pre:
                            wq = wq_pre[m]
                        else:
                            wq = wq_pool.tile([P, CI, P], F32R, tag="wq")
                            nc.sync.dma_start(wq[:], wqk[m, :, :].rearrange("p (ci n) -> p ci n", ci=CI))
                        for nt in range(TN):
                            ps = bpsum.tile([P, FN], F32)
                            for ci in range(CI):
                                nc.tensor.matmul(
                                    ps[:], wq[:, ci, :], xT[:, ci, nt * FN:(nt + 1) * FN],
                                    start=(ci == 0), stop=(ci == CI - 1),
                                )
                            st = bstage.tile([P, FN], F32R)
                            nc.vector.tensor_copy(st[:], ps[:])
                            nc.sync.dma_start(qkT_m[m][:, nt * FN:(nt + 1) * FN], st[:])
                wq_pool.release()

            # wpt prefetched here so phase D starts without a DMA stall;
            # the pool stays open through C + D.
            with (
                tc.tile_pool(name="wp", bufs=1) as wp_pool,
            ):
                wpt = wp_pool.tile([P, HPG, C], F32R)  # 64KB/partition resident
                # loaded in per-head 1MB chunks inside the C loop: one big
                # 8MB DMA would sit in a FIFO HW queue ahead of the phase C
                # prefetches and stall them

                # ---------- Phase C: causal attention per head ----------
                with (
                    tc.tile_pool(name="mk", bufs=1) as mk_pool,
                    tc.tile_pool(name="qkh", bufs=2) as qk_pool,
                    tc.tile_pool(name="vn", bufs=2) as vn_pool,
                    tc.tile_pool(name="cps", bufs=4, space="PSUM") as cps,
                    tc.tile_pool(name="rps", bufs=2, space="PSUM") as rps,
                    tc.tile_pool(name="ops", bufs=2, space="PSUM") as ops,
                    tc.tile_pool(name="pt", bufs=4) as pt_pool,
                    tc.tile_pool(name="rr", bufs=2) as rr_pool,
                ):
                    masks_sb = mk_pool.tile([P, 4, FN], F32)
                    nc.gpsimd.dma_start(masks_sb[:], masks[:, :, :])

                    for h in range(HPG):
                        qT = qk_pool.tile([P, T], F32R, tag="qT")
                        kT = qk_pool.tile([P, T], F32R, tag="kT")
                        nc.gpsimd.dma_start(qT[:], qkT_m[h][:, :])
                        nc.gpsimd.dma_start(kT[:], qkT_m[HPG + h][:, :])
                        vn = vn_pool.tile([P, TM, HD], F32R)  # V natural [k, d] chunks
                        nc.gpsimd.dma_start(
                            vn[:], v_h[h][:, :].rearrange("(k p) d -> p k d", p=P)
                        )
                        nc.sync.dma_start(wpt[:, h, :], wp[:, h, :])

                        # flat software pipeline over all causal (j, i)
                        # chunks of this head: the S matmul runs 3 iterations
                        # ahead so exp/round latency never stalls the PE
                        seq = [(j, i) for j in range(TN) for i in range(4 * j + 4)]
                        LOOK = 3

                        def emit_S(idx):
                            j, i = seq[idx]
                            psS = cps.tile([P, FN], F32, tag="psS", bufs=4)
                            nc.tensor.matmul(
                                psS[:], kT[:, i * P:(i + 1) * P],
                                qT[:, j * FN:(j + 1) * FN],
                                start=True, stop=True,
                            )
                            if i >= 4 * j:  # diagonal block: mask k > q
                                nc.vector.tensor_add(
                                    psS[:], psS[:], masks_sb[:, i - 4 * j, :]
                                )
                            return psS

                        psq = [emit_S(idx) for idx in range(min(LOOK, len(seq)))]
                        po = pr = None
                        for k, (j, i) in enumerate(seq):
                            if k + LOOK < len(seq):
                                psq.append(emit_S(k + LOOK))
                            psS_cur = psq.pop(0)
                            if i == 0:
                                po = ops.tile([P, FN], F32, tag="po", bufs=2)
                                pr = rps.tile([1, FN], F32, tag="pr", bufs=2)
                            nk = 4 * j + 4
                            ptt_f = pt_pool.tile([P, FN], F32, tag="pttf")
                            # P^T = exp(S^T * scale); logits ~ N(0,1) so no
                            # max-subtraction needed in fp32. ACT writes fp32
                            # (its fp32r path is ~2.6x slower); DVE rounds.
                            nc.scalar.activation(
                                ptt_f[:], psS_cur[:],
                                mybir.ActivationFunctionType.Exp, scale=SCALE,
                            )
                            ptt = pt_pool.tile([P, FN], F32R)
                            nc.vector.tensor_copy(ptt[:], ptt_f[:])
                            nc.tensor.matmul(
                                po[:], vn[:, i, :], ptt[:],
                                start=(i == 0), stop=(i == nk - 1),
                            )
                            nc.tensor.matmul(
                                pr[:], ones[:], ptt[:],
                                start=(i == 0), stop=(i == nk - 1),
                            )
                            if i != nk - 1:
                                continue
                            # normalize: O^T * exp(-ln r); the [1,512] row is
                            # replicated across partitions by a DMA with a
                            # partition-broadcast source AP, keeping all
                            # engines out of the replication
                            lnr = rr_pool.tile([1, FN], F32, tag="lnr")
                            nc.scalar.activation(
                                lnr[:], pr[:], mybir.ActivationFunctionType.Ln
                            )
                            rinv = rr_pool.tile([1, FN], F32, tag="rinv")
                            nc.scalar.activation(
                                rinv[:], lnr[:], mybir.ActivationFunctionType.Exp,
                                scale=-1.0,
                            )
                            rd = dram_pool.tile([1, FN], F32, name=f"rinv{h}_{j}")
                            nc.sync.dma_start(rd[:], rinv[:])
                            rb = rr_pool.tile([P, FN], F32, tag="rb")
                            nc.sync.dma_start(rb[:], rd[0:1, :].to_broadcast((P, FN)))
                            otj = rr_pool.tile([P, FN], F32, tag="otj")
                            nc.vector.tensor_mul(otj[:], po[:], rb[:])
                            nc.sync.dma_start(
                                oT[h * P:(h + 1) * P, j * FN:(j + 1) * FN], otj[:]
                            )

                # ---------- Phase D: out = O @ w_proj (partial over heads) ----------
                with (
                    tc.tile_pool(name="otm", bufs=3) as otm_pool,
                    tc.tile_pool(name="otr", bufs=3) as otr_pool,
                    tc.tile_pool(name="dps", bufs=4, space="PSUM") as dps,
                    tc.tile_pool(name="dstage", bufs=4) as dstage,
                ):
                    oT_re = oT[:, :].rearrange("(hh p) t -> p hh t", p=P)
                    for tm in range(TM):
                        otm = otm_pool.tile([P, HPG, P], F32)
                        nc.gpsimd.dma_start(otm[:], oT_re[:, :, tm * P:(tm + 1) * P])
                        otr = otr_pool.tile([P, HPG, P], F32R)
                        nc.vector.tensor_copy(otr[:], otm[:])
                        for n in range(C // FN):
                            ps = dps.tile([P, FN], F32)
                            for hh in range(HPG):
                                nc.tensor.matmul(
                                    ps[:], otr[:, hh, :], wpt[:, hh, n * FN:(n + 1) * FN],
                                    start=(hh == 0), stop=(hh == HPG - 1),
                                )
                            st = dstage.tile([P, FN], F32)
                            nc.vector.tensor_copy(st[:], ps[:])
                            nc.sync.dma_start(out[tm * P:(tm + 1) * P, n * FN:(n + 1) * FN], st[:])
    nc.compile()
    return nc


def _build_masks() -> np.ndarray:
    """Additive causal masks: masks[r, m, c] = 0.0 iff (c - r) >= 128*m
    else -1e9.

    S^T diagonal tile at k-chunk i, q-chunk j: entry (r, c) is valid
    (k <= q) iff 128*i + r <= 512*j + c, i.e. c - r >= 128*(i - 4*j).
    """
    rr = np.arange(P)[:, None, None]
    mm = np.arange(4)[None, :, None]
    cc = np.arange(FN)[None, None, :]
    valid = (cc - rr) >= P * mm
    return np.where(valid, 0.0, NEG).astype(np.float32)


_CACHE: dict = {}


def _get_nc() -> bass.Bass:
    if "nc" not in _CACHE:
        _CACHE["nc"] = build_nc()
    return _CACHE["nc"]


def _make_in_maps(x, w_qkv, w_proj):
    x = np.asarray(x, dtype=np.float32)
    w_qkv = np.asarray(w_qkv, dtype=np.float32)
    w_proj = np.asarray(w_proj, dtype=np.float32)
    masks = _build_masks()
    in_maps = []
    for core in range(N_CORES):
        b, g = divmod(core, G)
        wq = w_qkv[:, DG * g:DG * (g + 1)]
        wk = w_qkv[:, C + DG * g:C + DG * (g + 1)]
        wvs = w_qkv[:, 2 * C + DG * g:2 * C + DG * (g + 1)]
        w_qk = np.concatenate([wq, wk], axis=1)  # [C, 2048]
        # [ci*128+p, m*128+col] -> [m, p, ci*128+col]
        wqk_perm = np.ascontiguousarray(
            w_qk.reshape(CI, P, MQK, P).transpose(2, 1, 0, 3).reshape(MQK, P, CI * P)
        )
        # [ci*128+p, nv*512+vc] -> [nv, p, ci*512+vc]
        wv_perm = np.ascontiguousarray(
            wvs.reshape(CI, P, NV, FN).transpose(2, 1, 0, 3).reshape(NV, P, CI * FN)
        )
        wpg = w_proj[DG * g:DG * (g + 1), :]  # [1024, 2048]
        wp_perm = np.ascontiguousarray(
            wpg.reshape(HPG, P, C).transpose(1, 0, 2)  # [p, hh, n]
        )
        in_maps.append({
            "xt": np.ascontiguousarray(x[b].T),
            "wqk": wqk_perm,
            "wv": wv_perm,
            "wp": wp_perm,
            "masks": masks,
        })
    return in_maps


def run_spmd(x, w_qkv, w_proj, trace: bool = False):
    """Returns (out [B,T,C] fp32, BassKernelResults)."""
    in_maps = _make_in_maps(x, w_qkv, w_proj)
    kr = run_bass_kernel_spmd(_get_nc(), in_maps, list(range(N_CORES)), trace=trace)
    res = kr.results
    out = np.empty((B, T, C), dtype=np.float32)
    for b in range(B):
        out[b] = res[G * b]["out"] + res[G * b + 1]["out"]
    return out, kr


def kernel(x, w_qkv, w_proj) -> np.ndarray:
    out, _ = run_spmd(x, w_qkv, w_proj, trace=False)
    return out



# revision 13
# speedup vs baseline: 1.0816x; 1.0816x over previous
"""Causal self-attention on 8 TRN2 NeuronCores.

Sharding: 8 cores = 4 batches x 2 head-groups (data parallel on B,
tensor parallel on heads). Core (b, g) computes batch b, heads
8g..8g+7 end-to-end (qkv slice -> causal attention -> partial
projection); the host sums the two per-batch partials.

v2: all-bf16 dataflow. Everything stays SBUF-resident (q/k/v/O never
round-trip through DRAM), matmuls run bf16 (same PE rate as fp32r but
FWL halves the stationary-load cost and all DMA/SBUF footprints halve).
The softmax row-sum is accumulated with cheap DVE bf16 adds plus one
ones-matmul per q-block instead of a full PE pass per chunk, and the
1/r broadcast is a tiny PE matmul instead of a DRAM round-trip.

Self-contained: hardcodes B=4, T=2048, C=2048, H=16, HD=128.
"""

import numpy as np

import concourse.bass as bass
import concourse.mybir as mybir
import concourse.tile as tile
from concourse import bacc
from concourse.bass_utils import run_bass_kernel_spmd

B, T, C, H = 4, 2048, 2048, 16
HD = 128          # head dim
G = 2             # head groups (tensor parallel)
HPG = H // G      # 8 heads per core
DG = HPG * HD     # 1024 = per-core concat head dim
N_CORES = 8
SCALE = float(HD) ** -0.5
NEG = -1.0e9      # additive causal mask value

F32 = mybir.dt.float32
F32R = mybir.dt.float32r
BF16 = mybir.dt.bfloat16

P = 128           # partitions
FN = 512          # moving free-dim per matmul (one PSUM bank of fp32)
CI = C // P       # 16 contraction chunks over C
TM = T // P       # 16 t chunks of 128
TN = T // FN      # 4 t chunks of 512
MQK = 2 * DG // P # 16 qk^T row chunks
NV = DG // FN     # 2 v column chunks of 512
HC = CI // 2      # wv streamed in half-ci tiles


def build_nc() -> bass.Bass:
    nc = bacc.Bacc()
    # xt = x.T (host pre-transposed, bf16). wqk = [wq|wk] cols for this
    # core's heads, host-permuted to [m, p, ci*128+col]. wv = v cols,
    # host-permuted to [nv, p, ci*512+vc]. wp = w_proj rows, host-
    # permuted to [p, hh, n]. All weights bf16.
    xt = nc.declare_dram_parameter("xt", [C, T], BF16, isOutput=False)
    wqk = nc.declare_dram_parameter("wqk", [MQK, P, CI * P], BF16, isOutput=False)
    wv = nc.declare_dram_parameter("wv", [NV, P, CI * FN], BF16, isOutput=False)
    wp = nc.declare_dram_parameter("wp", [P, HPG, C], BF16, isOutput=False)
    masks = nc.declare_dram_parameter("masks", [P, 4, FN], F32, isOutput=False)
    out = nc.declare_dram_parameter("out", [T, C], F32, isOutput=True)

    with tile.TileContext(nc) as tc:
        with (
            tc.tile_pool(name="consts", bufs=1) as consts,
            tc.tile_pool(name="dram", bufs=1, space="DRAM") as dram_pool,
        ):
            ones_f = consts.tile([P, 1], F32)
            nc.gpsimd.memset(ones_f[:], 1.0)
            ones_bf = consts.tile([P, 1], BF16)
            nc.scalar.copy(ones_bf[:], ones_f[:])
            masks_sb = consts.tile([P, 4, FN], F32)
            nc.sync.dma_start(masks_sb[:], masks[:, :, :])
            # warm the exp activation table while ACT is idle
            warm = consts.tile([1, 1], F32)
            nc.scalar.activation(
                warm[:], ones_f[0:1, :], mybir.ActivationFunctionType.Exp
            )

            with (
                tc.tile_pool(name="qkT", bufs=1) as qkT_pool,
                tc.tile_pool(name="vn", bufs=1) as vn_pool,
            ):
                # qk^T resident: m 0..7 = q^T per head, 8..15 = k^T per head
                qkT = [qkT_pool.tile([P, T], BF16, name=f"qkT{m}") for m in range(MQK)]
                # V natural chunks: [t-within-chunk, head, tm, d]
                vn = vn_pool.tile([P, HPG, TM, HD], BF16)

                with tc.tile_pool(name="xT", bufs=1) as xT_pool:
                    xT = xT_pool.tile([P, CI, T], BF16)  # x^T resident 64KB/part

                    # ---------- Phase A: qk^T = wqk.T @ x.T ----------
                    # xT streamed by t-halves so the first m-sweep starts
                    # ~2us in instead of waiting for the full x load.
                    with (
                        tc.tile_pool(name="wq", bufs=3) as wq_pool,
                        tc.tile_pool(name="aps", bufs=4, space="PSUM") as apsum,
                    ):
                        for half in range(2):
                            t0, t1 = half * (T // 2), (half + 1) * (T // 2)
                            for ci in range(CI):
                                nc.sync.dma_start(
                                    xT[:, ci, t0:t1], xt[ci * P:(ci + 1) * P, t0:t1]
                                )
                            for m in range(MQK):
                                wq = wq_pool.tile([P, CI, P], BF16, tag="wq")
                                nc.sync.dma_start(
                                    wq[:],
                                    wqk[m, :, :].rearrange("p (ci n) -> p ci n", ci=CI),
                                )
                                for nt in range(2 * half, 2 * half + 2):
                                    ps = apsum.tile([P, FN], F32)
                                    for ci in range(CI):
                                        nc.tensor.matmul(
                                            ps[:], wq[:, ci, :],
                                            xT[:, ci, nt * FN:(nt + 1) * FN],
                                            start=(ci == 0), stop=(ci == CI - 1),
                                        )
                                    if nt % 2 == 0:
                                        nc.vector.tensor_copy(
                                            qkT[m][:, nt * FN:(nt + 1) * FN], ps[:]
                                        )
                                    else:
                                        nc.scalar.copy(
                                            qkT[m][:, nt * FN:(nt + 1) * FN], ps[:]
                                        )

                    # ---------- Phase B: V = x @ wv ----------
                    with (
                        tc.tile_pool(name="wvp", bufs=3) as wv_pool,
                        tc.tile_pool(name="bps", bufs=4, space="PSUM") as bpsum,
                    ):
                        for nv in range(NV):
                            wvt = {}
                            for half in range(2):
                                t = wv_pool.tile([P, HC, FN], BF16, tag="wvt")
                                nc.sync.dma_start(
                                    t[:],
                                    wv[nv, :, half * HC * FN:(half + 1) * HC * FN]
                                    .rearrange("p (ci n) -> p ci n", ci=HC),
                                )
                                wvt[half] = t
                            for tm in range(TM):
                                ps = bpsum.tile([P, FN], F32)
                                for ci in range(CI):
                                    nc.tensor.matmul(
                                        ps[:], xT[:, ci, tm * P:(tm + 1) * P],
                                        wvt[ci // HC][:, ci % HC, :],
                                        start=(ci == 0), stop=(ci == CI - 1),
                                    )
                                if tm % 2 == 0:
                                    nc.vector.tensor_copy(ps_dest_vn(vn, nv, tm), ps[:])
                                else:
                                    nc.scalar.copy(ps_dest_vn(vn, nv, tm), ps[:])

                # xT freed; wpt + oT live in its space through C..D
                with (
                    tc.tile_pool(name="wp", bufs=1) as wp_pool,
                    tc.tile_pool(name="oTp", bufs=1) as oT_pool,
                ):
                    wpt = wp_pool.tile([P, HPG, C], BF16)
                    for hh in range(HPG):
                        nc.sync.dma_start(wpt[:, hh, :], wp[:, hh, :])
                    oT = oT_pool.tile([P, HPG, T], BF16)  # attention out^T

                    # ---------- Phase C: causal attention per head ----------
                    with (
                        tc.tile_pool(name="cps", bufs=2, space="PSUM") as cps,
                        tc.tile_pool(name="ops", bufs=2, space="PSUM") as ops,
                        tc.tile_pool(name="prs", bufs=1, space="PSUM") as prs,
                        tc.tile_pool(name="pt", bufs=3) as pt_pool,
                        tc.tile_pool(name="rr", bufs=2) as rr_pool,
                    ):
                        for h in range(HPG):
                            qT = qkT[h]
                            kT = qkT[HPG + h]
                            # pair-slots: (j, p) with 2 k-chunks per slot
                            slots = [(j, p) for j in range(TN)
                                     for p in range(2 * j + 2)]

                            def emit_S(idx):
                                j, p = slots[idx]
                                psS = cps.tile([P, 2, FN], F32, tag="psS")
                                for c in range(2):
                                    i = 2 * p + c
                                    nc.tensor.matmul(
                                        psS[:, c, :], kT[:, i * P:(i + 1) * P],
                                        qT[:, j * FN:(j + 1) * FN],
                                        start=True, stop=True,
                                    )
                                if p >= 2 * j:  # diagonal pair: mask k > q
                                    mc = 2 * (p - 2 * j)
                                    nc.vector.tensor_add(
                                        psS[:, :, :], psS[:, :, :],
                                        masks_sb[:, mc:mc + 2, :],
                                    )
                                return psS

                            LOOK = 1
                            psq = [emit_S(i) for i in range(min(LOOK, len(slots)))]
                            po = None
                            racc = None
                            for k, (j, p) in enumerate(slots):
                                if k + LOOK < len(slots):
                                    psq.append(emit_S(k + LOOK))
                                psS_cur = psq.pop(0)
                                ptt = pt_pool.tile([P, 2, FN], BF16, tag="ptt")
                                nc.scalar.activation(
                                    ptt[:, :, :], psS_cur[:, :, :],
                                    mybir.ActivationFunctionType.Exp, scale=SCALE,
                                )
                                if p == 0:
                                    po = ops.tile([P, FN], F32, tag="po")
                                    racc = rr_pool.tile([P, FN], BF16, tag="racc")
                                    nc.vector.tensor_copy(racc[:], ptt[:, 0, :])
                                else:
                                    nc.vector.tensor_add(
                                        racc[:], racc[:], ptt[:, 0, :]
                                    )
                                nc.vector.tensor_add(racc[:], racc[:], ptt[:, 1, :])
                                nk = 2 * j + 2
                                for c in range(2):
                                    i = 2 * p + c
                                    nc.tensor.matmul(
                                        po[:], vn[:, h, i, :], ptt[:, c, :],
                                        start=(p == 0 and c == 0),
                                        stop=(p == nk - 1 and c == 1),
                                    )
                                if p != nk - 1:
                                    continue
                                # normalize: O^T[:, jblk] = po * (1/r) with the
                                # row 1/r broadcast across partitions by a tiny
                                # rank-1 matmul (ones_col x rinv)
                                pr = prs.tile([1, FN], F32, tag="pr")
                                nc.tensor.matmul(
                                    pr[:], ones_bf[:], racc[:],
                                    start=True, stop=True,
                                )
                                rinv = rr_pool.tile([1, FN], F32, tag="rinv")
                                nc.vector.reciprocal(rinv[:], pr[:])
                                # broadcast 1/r across partitions via a DMA
                                # bounce (keeps all engines out of it)
                                rd = dram_pool.tile([1, FN], F32, name=f"rv{h}_{j}")
                                nc.sync.dma_start(rd[:], rinv[:])
                                rb = rr_pool.tile([P, FN], F32, tag="rb")
                                nc.sync.dma_start(
                                    rb[:], rd[0:1, :].to_broadcast((P, FN))
                                )
                                nc.vector.tensor_mul(
                                    oT[:, h, j * FN:(j + 1) * FN], po[:], rb[:]
                                )

                    # ---------- Phase D: out = O @ w_proj (partial) ----------
                    with (
                        tc.tile_pool(name="dps", bufs=4, space="PSUM") as dps,
                        tc.tile_pool(name="dstage", bufs=4) as dstage,
                    ):
                        for tm in range(TM):
                            for n in range(C // FN):
                                ps = dps.tile([P, FN], F32)
                                for hh in range(HPG):
                                    nc.tensor.matmul(
                                        ps[:], oT[:, hh, tm * P:(tm + 1) * P],
                                        wpt[:, hh, n * FN:(n + 1) * FN],
                                        start=(hh == 0), stop=(hh == HPG - 1),
                                    )
                                st = dstage.tile([P, FN], F32)
                                if n % 2 == 0:
                                    nc.vector.tensor_copy(st[:], ps[:])
                                else:
                                    nc.scalar.copy(st[:], ps[:])
                                nc.sync.dma_start(
                                    out[tm * P:(tm + 1) * P, n * FN:(n + 1) * FN],
                                    st[:],
                                )
    nc.compile()
    return nc


def ps_dest_vn(vn, nv, tm):
    # psum [128, 512] covers 4 heads' d-columns for this (nv, tm)
    return vn[:, 4 * nv:4 * (nv + 1), tm, :]


def _build_masks() -> np.ndarray:
    """Additive causal masks: masks[r, m, c] = 0.0 iff (c - r) >= 128*m
    else -1e9 (S^T diagonal tile at k-chunk i, q-chunk j: valid iff
    c - r >= 128*(i - 4*j))."""
    rr = np.arange(P)[:, None, None]
    mm = np.arange(4)[None, :, None]
    cc = np.arange(FN)[None, None, :]
    valid = (cc - rr) >= P * mm
    return np.where(valid, 0.0, NEG).astype(np.float32)


_CACHE: dict = {}


def _get_nc() -> bass.Bass:
    if "nc" not in _CACHE:
        _CACHE["nc"] = build_nc()
    return _CACHE["nc"]


def _make_in_maps(x, w_qkv, w_proj):
    import ml_dtypes

    bf16 = ml_dtypes.bfloat16
    x = np.asarray(x, dtype=np.float32)
    w_qkv = np.asarray(w_qkv, dtype=np.float32)
    w_proj = np.asarray(w_proj, dtype=np.float32)
    masks = _build_masks()
    in_maps = []
    for core in range(N_CORES):
        b, g = divmod(core, G)
        wq = w_qkv[:, DG * g:DG * (g + 1)]
        wk = w_qkv[:, C + DG * g:C + DG * (g + 1)]
        wvs = w_qkv[:, 2 * C + DG * g:2 * C + DG * (g + 1)]
        w_qk = np.concatenate([wq, wk], axis=1)  # [C, 2048]
        # [ci*128+p, m*128+col] -> [m, p, ci*128+col]
        wqk_perm = np.ascontiguousarray(
            w_qk.reshape(CI, P, MQK, P).transpose(2, 1, 0, 3).reshape(MQK, P, CI * P)
        ).astype(bf16)
        # [ci*128+p, nv*512+vc] -> [nv, p, ci*512+vc]
        wv_perm = np.ascontiguousarray(
            wvs.reshape(CI, P, NV, FN).transpose(2, 1, 0, 3).reshape(NV, P, CI * FN)
        ).astype(bf16)
        wpg = w_proj[DG * g:DG * (g + 1), :]  # [1024, 2048]
        wp_perm = np.ascontiguousarray(
            wpg.reshape(HPG, P, C).transpose(1, 0, 2)  # [p, hh, n]
        ).astype(bf16)
        in_maps.append({
            "xt": np.ascontiguousarray(x[b].T).astype(bf16),
            "wqk": wqk_perm,
            "wv": wv_perm,
            "wp": wp_perm,
            "masks": masks,
        })
    return in_maps


def run_spmd(x, w_qkv, w_proj, trace: bool = False):
    """Returns (out [B,T,C] fp32, BassKernelResults)."""
    in_maps = _make_in_maps(x, w_qkv, w_proj)
    kr = run_bass_kernel_spmd(_get_nc(), in_maps, list(range(N_CORES)), trace=trace)
    res = kr.results
    out = np.empty((B, T, C), dtype=np.float32)
    for b in range(B):
        out[b] = res[G * b]["out"] + res[G * b + 1]["out"]
    return out, kr


def kernel(x, w_qkv, w_proj) -> np.ndarray:
    out, _ = run_spmd(x, w_qkv, w_proj, trace=False)
    return out
